# revision 53
# baseline (speedup 1.0000x reference)
"""Trainium2 Bass kernel for nn_CVFRLayer (recurrent attractor scan).

x_{t+1} = (1-dt)*x_t + nl(x_t) @ B' + z_t,   nl(x) = x^2/(gamma+x^2)
  B' = dt*(A@(I-P) + P).T  (P block-diagonal projector, computed host-side O(n^2))
  z_t = noise_t @ (sqrt(dt)*eps*G.T)

Strategy: pure data parallel over 8 NeuronCores, 64 batch rows per core.
State kept in a "folded" layout [128, 1024]: partitions 0-63 hold features
0-1023, partitions 64-127 hold features 1024-2047 for the same 64 batch rows.

The scan matmul runs in fp16 as column-tiled pairs (tile_position
(0,0)/(0,64)) so two M=64 matmuls occupy the full 128x128 PE array
concurrently (~216ns/pair).  The noise projections run in fp8-e4m3 with
perf_mode=DoubleRow (2 fp8 MACs per PE cell per cycle, ~2x fp16 throughput;
DoubleRow + column tiling is ISA-illegal — s3d3_mm_valid_dst_partition — so
the scan pair itself cannot use it) as full-M=128 matmuls covering 2 steps
per noise "supertile", interleaved half-a-supertile per scan step and gated
to the current step by a value-preserving 1-element ntt rewrite that reads x
(otherwise the tile scheduler, whose DoubleRow cost model underestimates 2x,
lumps whole supertiles into one step's shadow and starves alternate steps).

The per-step serial tail is: x-update (DVE) -> nl (ACT square + DVE
recip chain) -> per-chunk xbar transpose (SP queue) producing fT in a
[128, 8, 128] layout whose k-tile kt is the slice [:, kt%8, 64*(kt//8):+64];
chunk c0's transpose covers K_ORDER's first 8 k-tiles so the next step
starts while chunk c1's epilogue is still in flight.  All noise PSUM
copy-outs run on ACT (keeping DVE clear for the tail); the partition-aligned
half of each copy-out goes straight into the per-step zf tile and only one
[64,1024] partition-crossing DMA per step remains (SBUF DMAs share global
round-robin semaphore lanes; a slow DMA there stalls the transposes).
"""

import math
import sys

if "/opt/trn_rl_repo" not in sys.path:
    sys.path.insert(0, "/opt/trn_rl_repo")

import numpy as np

SIZE = 2048
N_CLASSES = 16
STEPS = 100
DT = 0.03
GAMMA = 0.125
BETA = 1.0
EIG = 1.0
EPSILON = 0.1

N_CORES = 8
BPC = 64  # batch rows per core
HALF = SIZE // 2  # folded free dim
KT = SIZE // 128  # 16 contraction tiles
A_COEF = 1.0 - DT
CHUNKS = [(0, 512), (512, 1024)]  # folded-col chunks per step

# fp8-e4m3 path: quantization scales (powers of 2; products must match the
# PSUM descales baked below).  max|gt| ~ 3.9e-4 * 2^19 = 201 < 240;
# max|noise| ~ 5.5 * 2^5 = 176 < 240.
NOISE_FP8 = True
S_G = 2.0**19
S_N = 2.0**5
INV_SN_SG = 1.0 / (S_N * S_G)

_cache = {}


def _build(steps):
    import concourse.bacc as bacc
    import concourse.mybir as mybir
    import concourse.tile as tile

    f16 = mybir.dt.float16
    f32 = mybir.dt.float32
    f8 = mybir.dt.float8e4
    AF = mybir.ActivationFunctionType
    OP = mybir.AluOpType
    PM = mybir.MatmulPerfMode

    n_st = steps // 2  # noise supertiles (2 steps each)
    assert steps % 2 == 0

    ndt = f8 if NOISE_FP8 else f16

    nc = bacc.Bacc("TRN2", target_bir_lowering=False, debug=False, num_devices=N_CORES)

    # register the tiny Ln bias const (only 0.0/1.0 are pre-registered)
    _bias_t = nc.alloc_sbuf_tensor("const-float32-1e-35", [128, 1], f32)
    nc.gpsimd.memset(_bias_t.ap(), 1e-35)
    nc.const_aps.aps[(f32, 1e-35)] = _bias_t.ap()
    nc.all_engine_barrier()

    x0_d = nc.declare_dram_parameter("x0", [128, HALF], f32, isOutput=False)
    bt_d = nc.declare_dram_parameter("bt", [SIZE, SIZE], f16, isOutput=False)
    gt_d = nc.declare_dram_parameter("gt", [SIZE, SIZE], ndt, isOutput=False)
    nt_d = nc.declare_dram_parameter("nt", [SIZE, steps * BPC], ndt, isOutput=False)
    out_d = nc.declare_dram_parameter("out", [128, HALF], f32, isOutput=True)

    # 3D views with the 128-partition dim first: [(k p) n -> p k n]
    bt_v = bt_d.rearrange("(k p) n -> p k n", p=128)
    gt_v = gt_d.rearrange("(k p) n -> p k n", p=128)
    nt_v = nt_d.rearrange("(k p) n -> p k n", p=128)

    with tile.TileContext(nc) as tc:
        with (
            tc.tile_pool(name="const", bufs=1) as constp,
            tc.tile_pool(name="state", bufs=1) as statep,
            tc.tile_pool(name="f16t", bufs=4) as f16p,
            tc.tile_pool(name="fT", bufs=4) as fTp,
            tc.tile_pool(name="zu", bufs=3) as zup,
            tc.tile_pool(name="zf", bufs=8) as zfp,
            tc.tile_pool(name="nt", bufs=4) as ntp,
            tc.tile_pool(name="scr", bufs=2) as scrp,
            tc.tile_pool(name="sps", bufs=4, space="PSUM") as spsp,
            tc.tile_pool(name="nps", bufs=4, space="PSUM") as npsp,
        ):
            # ---- persistent tiles ----
            bt = constp.tile([128, KT, SIZE], f16, tag="bt")
            gt = constp.tile([128, KT, SIZE], ndt, tag="gt")
            x = statep.tile([128, HALF], f32, tag="x")
            w = statep.tile([128, HALF], f32, tag="w")

            # queue order matters (single regular-DMA queue): lead noise tiles
            # first (small, needed immediately), then gt chunks, x0, bt last

            def nl_chunk(src_ap, dst_ap, wd):
                """dst(f16) = nl(src) = src^2/(gamma+src^2) for [128, wd]."""
                s = scrp.tile([128, 512], f32, tag="s")
                d = scrp.tile([128, 512], f32, tag="d")
                r = scrp.tile([128, 512], f32, tag="r")
                nc.scalar.activation(s[:, :wd], src_ap, AF.Square)
                nc.vector.tensor_scalar_add(d[:, :wd], s[:, :wd], GAMMA)
                nc.vector.reciprocal_approx_fast(r[:, :wd], d[:, :wd])
                nc.vector.tensor_scalar(dst_ap, r[:, :wd], -GAMMA, 1.0, OP.mult, OP.add)

            def transpose_chunk(f16t, fT_next, c0, c1):
                # per-chunk xbar transpose (~1.27us flat, on the SP queue):
                # f16t[:, c0:c1] -> fT_next[:, c0/128 : c1/128, :].  Output
                # partition = feature%128, dim1 = (feature%1024)//128, dim2 =
                # folded row r = 64*(feature//1024) + batch.  The scan
                # k-tile kt is the slice [:, kt%8, 64*(kt//8) : 64*(kt//8)+64],
                # so chunk c0's transpose yields k-tiles {0-3, 8-11} — exactly
                # the first 8 consumed by K_ORDER — and runs while chunk c1
                # still computes; c1's transpose hides under the next step's
                # first 8 scan pairs.  No transpose sits on the serial tail.
                nc.sync.dma_start_transpose(
                    fT_next[:, c0 // 128 : c1 // 128, :], f16t[:, c0:c1]
                )

            nt_tiles = {}

            def nt_prefetch(st, eng=None):
                # the `st not in nt_tiles` guard prevents re-fetching a tile
                # that was already explicitly prefetched (a duplicate fetch
                # lands at the BACK of the DMA queue, behind all the weight
                # loads, and stalls the lead noise work ~40us at startup)
                if st < n_st and st not in nt_tiles:
                    ntt = ntp.tile([128, KT, 128], ndt, tag="nt")
                    (eng or nc.scalar).dma_start(
                        ntt[:], nt_v[:, :, st * 128 : (st + 1) * 128]
                    )
                    nt_tiles[st] = ntt

            nc.scalar.dma_start(x[:], x0_d[:])
            # lead noise inputs ahead of the initial transposes on the SP
            # queue (tiny; the first noise matmuls need them ~10us in)
            nt_prefetch(0, eng=nc.sync)
            nt_prefetch(1, eng=nc.sync)
            nc.sync.dma_start(gt[:, :, 0:512], gt_v[:, :, 0:512])

            # ---- initial f(x0) ----
            # Emitted BEFORE the weight loads: the ACT queue is FIFO and
            # also carries every DMA trigger, so emitting nl(x0) after the
            # loads would queue its Square op behind ~20 triggers (~40us).
            fT_cur = fTp.tile([128, 8, 128], f16, tag="fT")
            f16t0 = f16p.tile([128, HALF], f16, tag="f16t")
            for c0, c1 in CHUNKS:
                nl_chunk(x[:, c0:c1], f16t0[:, c0:c1], c1 - c0)
                transpose_chunk(f16t0, fT_cur, c0, c1)

            # weight loads split across BOTH HWDGE queues for parallel DMA
            # bandwidth at startup: noise-path tensors (nt, gt) on the SP
            # queue (behind the two initial transposes, ready ~7us), scan
            # bt k-tiles on the ACT queue in the K_ORDER the scan consumes
            # (per-k [128, 2048] contiguous slabs -> scan starts k-paced).
            for n in range(1, 4):
                nc.sync.dma_start(
                    gt[:, :, n * 512 : (n + 1) * 512],
                    gt_v[:, :, n * 512 : (n + 1) * 512],
                )
            for k in (0, 1, 2, 3, 8, 9, 10, 11, 4, 5, 6, 7, 12, 13, 14, 15):
                nc.scalar.dma_start(bt[:, k, :], bt_v[:, k, :])

            zf_tiles = {}
            zu_tiles = {}


            def noise_half(st, half):
                """Half a noise supertile: feature chunks [2*half, 2*half+2).

                Supertile st projects noise rows [128*st, 128*st+128) -> z for
                steps 2st, 2st+1. Split in two halves so every scan step gets
                ~7us of independent PE work to hide its serial epilogue tail.
                """
                if half == 0:
                    ntt = nt_tiles.pop(st)
                    # zuX: scratch for the partition-CROSSING quarter of each
                    # step's z (engines cannot cross partitions; only DMAs
                    # can).  The partition-aligned quarters are copied out of
                    # PSUM straight into the per-step zf tiles, so only ONE
                    # [64,1024] crossing DMA per step remains (fewer + faster
                    # DMAs keeps the shared DMAHW semaphore lanes flowing; a
                    # slow fold there stalls the fT transposes ~4us/step).
                    zuX = zup.tile([128, HALF], f16, tag="zu")
                    zfA = zfp.tile([128, HALF], f16, tag="zf")
                    zfB = zfp.tile([128, HALF], f16, tag="zf")
                    zu_tiles[st] = (ntt, zuX, zfA, zfB)
                    zf_tiles[2 * st] = zfA
                    zf_tiles[2 * st + 1] = zfB
                    # prefetch two supertiles ahead (4 bufs: st in use +
                    # st+1 + st+2 + one spare) so the fetch is never on the
                    # consumption path even when a DMA lane hiccups
                    nt_prefetch(st + 1)
                    nt_prefetch(st + 2)
                else:
                    ntt, zuX, zfA, zfB = zu_tiles.pop(st)
                # Scheduling gate: a value-preserving 1-element rewrite of ntt
                # that (transitively) READS x at the current step's version:
                #   zero8 = 0 * x[0,0]          (fp32 -> fp8 scratch)
                #   ntt[0,0,0] = ntt[0,0,0] + zero8
                # This makes this half's dk=0 matmuls (which read the
                # rewritten cell) data-dependent on the current scan step, so
                # the tile scheduler cannot pull the half earlier and lump
                # whole supertiles into one step's shadow (its DR cost model
                # underestimates 2x, which otherwise starves alternate steps
                # of PE filler work).
                zero8 = scrp.tile([1, 1], ndt, tag="zero8")
                nc.vector.tensor_scalar_mul(zero8[0:1, 0:1], x[0:1, 0:1], 0.0)
                nc.vector.tensor_tensor(
                    ntt[0:1, 0, 0:1], ntt[0:1, 0, 0:1], zero8[0:1, 0:1], OP.add
                )
                for n in (2 * half, 2 * half + 1):
                    ps = npsp.tile([128, 512], f32, tag="nps")
                    if NOISE_FP8:
                        # DoubleRow: contract two 128-k planes per instruction
                        # (fp8 pairs packed per PE cell -> 2 MACs/cell/cycle)
                        for dk in range(KT // 2):
                            nc.tensor.matmul(
                                ps[:],
                                ntt[:, 2 * dk : 2 * dk + 2, :],
                                gt[:, 2 * dk : 2 * dk + 2, n * 512 : (n + 1) * 512],
                                start=(dk == 0),
                                stop=(dk == KT // 2 - 1),
                                perf_mode=PM.DoubleRow,
                            )
                    else:
                        for k in range(KT):
                            nc.tensor.matmul(
                                ps[:],
                                ntt[:, k, :],
                                gt[:, k, n * 512 : (n + 1) * 512],
                                start=(k == 0),
                                stop=(k == KT - 1),
                            )
                    # copy out (fp32 psum -> fp16 sbuf, descale the fp8
                    # quantization scales).  PSUM rows 0:64 belong to step
                    # 2st, rows 64:128 to step 2st+1; chunks 0,1 are fold
                    # col-half 0 (zf partitions 0:64), chunks 2,3 col-half 1
                    # (zf partitions 64:128).  One half of each chunk is
                    # partition-ALIGNED with its zf destination (engine copy
                    # direct); the other goes to zuX scratch for the per-step
                    # crossing DMA below.  Alternate engines for balance.
                    dsc = INV_SN_SG if NOISE_FP8 else 1.0
                    c = (n % 2) * 512
                    if half == 0:
                        a_dst, a_src = zfA[0:64, c : c + 512], ps[0:64, :]
                        s_dst, s_src = zuX[64:128, c : c + 512], ps[64:128, :]
                    else:
                        a_dst, a_src = zfB[64:128, c : c + 512], ps[64:128, :]
                        s_dst, s_src = zuX[0:64, c : c + 512], ps[0:64, :]
                    # both halves on ACT: the DVE carries the per-step
                    # critical chain (x/w updates + nl) — keep it clear
                    nc.scalar.mul(a_dst, a_src, dsc)
                    nc.scalar.mul(s_dst, s_src, dsc)
                # one [64,1024] crossing DMA per half
                if half == 0:
                    nc.scalar.dma_start(zfB[0:64, :], zuX[64:128, :])
                else:
                    nc.scalar.dma_start(zfA[64:128, :], zuX[0:64, :])

            # lead: five halves queued before the scan so the PE always has
            # noise work buffered while a scan step's serial epilogue (nl +
            # transpose) runs. h0 halves first: they only read gt cols
            # 0-1024, which arrive well before cols 1024-2048 in the DMA
            # queue, so the PE starts earlier at startup.
            noise_half(0, 0)
            noise_half(1, 0)
            noise_half(0, 1)
            noise_half(1, 1)

            # scan k-tile consumption order: chunk c0's transpose produces
            # fT k-tiles {0-3, 8-11}; chunk c1's produces {4-7, 12-15}.
            # Consume c0's first so the next step starts while the previous
            # step's c1 transpose is still in flight.
            K_ORDER = [0, 1, 2, 3, 8, 9, 10, 11, 4, 5, 6, 7, 12, 13, 14, 15]

            # ---- the scan ----
            for t in range(steps):
                zf = zf_tiles.pop(t)
                # w = (1-dt)*x + z_t   (runs on DVE while PE does the matmuls)
                nc.vector.scalar_tensor_tensor(w[:], x[:], A_COEF, zf[:], OP.mult, OP.add)
                fT_next = fTp.tile([128, 8, 128], f16, tag="fT")
                f16t = f16p.tile([128, HALF], f16, tag="f16t")
                for c0, c1 in CHUNKS:
                    wd = c1 - c0
                    ps = spsp.tile([128, 512], f32, tag="sps")
                    for i, k in enumerate(K_ORDER):
                        fTk = fT_cur[:, k % 8, 64 * (k // 8) : 64 * (k // 8) + 64]
                        nc.tensor.matmul(
                            ps[0:64, :wd],
                            fTk,
                            bt[:, k, c0:c1],
                            start=(i == 0),
                            stop=(i == KT - 1),
                            tile_position=(0, 0),
                        )
                        nc.tensor.matmul(
                            ps[64:128, :wd],
                            fTk,
                            bt[:, k, HALF + c0 : HALF + c1],
                            start=(i == 0),
                            stop=(i == KT - 1),
                            tile_position=(0, 64),
                        )
                    # x_new = y + w
                    nc.vector.tensor_add(x[:, c0:c1], ps[:, :wd], w[:, c0:c1])
                    if t < steps - 1:
                        # the last step's fT has no consumer
                        nl_chunk(x[:, c0:c1], f16t[:, c0:c1], wd)
                        transpose_chunk(f16t, fT_next, c0, c1)
                fT_cur = fT_next
                # interleave noise production at consumption rate: half a
                # supertile (~3.5us of PE work with fp8 DoubleRow) after
                # every scan step, 5 halves ahead of consumption
                hi = t + 4  # linear half index 2*st + half
                st, half = hi // 2, hi % 2
                if st < n_st:
                    noise_half(st, half)

            nc.scalar.dma_start(out_d[:], x[:])

    nc.compile()
    return nc


def _q8(v, scale):
    import ml_dtypes

    return np.clip(v * scale, -240.0, 240.0).astype(ml_dtypes.float8_e4m3)


def _prepare_host(x, A, G, noise, steps):
    """Host-side O(n^2) weight prep + per-core input shards."""
    block = SIZE // N_CLASSES
    P = np.zeros((SIZE, SIZE), dtype=np.float32)
    for c in range(N_CLASSES):
        P[c * block : (c + 1) * block, c * block : (c + 1) * block] = 1.0 / block
    Ab = A.reshape(SIZE, N_CLASSES, block).mean(axis=2)
    A_P = np.repeat(Ab, block, axis=1)  # A @ P
    M0 = A - A_P + EIG * P  # A @ (I-P) + P
    bt_np = np.ascontiguousarray((DT * BETA) * M0.T).astype(np.float16)
    gt_f32 = (math.sqrt(DT) * EPSILON) * G.T
    if NOISE_FP8:
        gt_np = np.ascontiguousarray(_q8(gt_f32, S_G))
    else:
        gt_np = np.ascontiguousarray(gt_f32).astype(np.float16)

    in_maps = []
    for c in range(N_CORES):
        xs = x[c * BPC : (c + 1) * BPC]
        x0f = np.concatenate([xs[:, :HALF], xs[:, HALF:]], axis=0)
        x0f = np.ascontiguousarray(x0f, dtype=np.float32)
        nsh = noise[:steps, c * BPC : (c + 1) * BPC, :].reshape(steps * BPC, SIZE)
        if NOISE_FP8:
            nt_np = np.ascontiguousarray(_q8(nsh.T, S_N))
        else:
            nt_np = np.ascontiguousarray(nsh.astype(np.float16).T)
        in_maps.append({"x0": x0f, "bt": bt_np, "gt": gt_np, "nt": nt_np})
    return in_maps


def _run(in_maps, steps, trace=False):
    from concourse.bass_utils import run_bass_kernel_spmd

    key = steps
    if key not in _cache:
        _cache[key] = _build(steps)
    nc = _cache[key]
    res = run_bass_kernel_spmd(nc, in_maps, list(range(N_CORES)), trace=trace)
    outs = []
    for c in range(N_CORES):
        of = res.results[c]["out"]
        outs.append(np.concatenate([of[0:64, :], of[64:128, :]], axis=1))
    return np.concatenate(outs, axis=0).astype(np.float32), res


def kernel(x, A, G, noise):
    x = np.asarray(x, dtype=np.float32)
    A = np.asarray(A, dtype=np.float32)
    G = np.asarray(G, dtype=np.float32)
    noise = np.asarray(noise, dtype=np.float32)
    steps = noise.shape[0]
    in_maps = _prepare_host(x, A, G, noise, steps)
    out, _ = _run(in_maps, steps)
    return out

